# revision 6
# baseline (speedup 1.0000x reference)
"""ContrastivePatchLoss TRN2 kernel.

Math (reference): anchors = patches of main_out [512, 64, 256]; sims
against a 2048-entry bank (neg bank normally; pos bank only when a
patch's label-mean < 0.1, which for uniform [0,1) labels is a >40-sigma
event); softmax-style loss vs the ema positive pair; scalar mean.

Sharding: batch element b -> core b (8 cores, 64 patches = 4096 anchor
rows each). Banks replicated. Each core returns its 4096 per-row
log-fracs; host sums and negates.

Per-core pipeline (all engines overlapped, per 128-row tile):
  PE   : sims[128, 2048] = A_chunk.T @ bank (fp32r, 8 matmuls)
  DVE  : negated subsample row-max (stride-4) -> -m~  (safe exp shift)
  ACT  : exp(sims - m~) in-place in PSUM, accum_out = row-sums S
  DVE  : pos_sim via tensor_tensor_reduce on row-major A,2*ema tiles
Epilogue identity: with u = exp(pos - m~),
  frac = u / (u + S*(1+eps))   == exp(pos)/(sum_bank exp(s) * (1+eps) + exp(pos))
which matches the reference's frac with m' = exact bank log-sum-exp;
the only difference vs m=rowmax is the eps*e^m term, a <=~1e-5 relative
perturbation of the denominator. loss_row = -log(frac + eps).
"""

import numpy as np

B, C, H, W = 8, 256, 64, 64
PATCH = 8
TEMP = 0.5
EPS = 1e-5
L = 32
R = H * W            # anchor rows per core (64 patches x 64 positions)
NBANK = L * (H // PATCH) * (W // PATCH)   # 2048
M_TILES = R // 128   # 32
N_CORES = 8

_PROGRAM = None
TRACE = False
LAST_EXEC_NS = None
import os as _os

_EXP_INPLACE = _os.environ.get("K_EXP_INPLACE", "1") == "1"
_STRIDED_MAX = _os.environ.get("K_STRIDED_MAX", "1") == "1"


def _build_program():
    import concourse.tile as tile
    from concourse import bacc, mybir

    F = mybir.ActivationFunctionType
    Alu = mybir.AluOpType
    X = mybir.AxisListType.X
    f32 = mybir.dt.float32
    f32r = mybir.dt.float32r

    nc = bacc.Bacc(None)
    a_cm = nc.declare_dram_parameter("a_cm", [C, R], f32r, isOutput=False)
    at_rm = nc.declare_dram_parameter("at_rm", [R, C], f32, isOutput=False)
    pt_rm = nc.declare_dram_parameter("pt_rm", [R, C], f32, isOutput=False)
    nb = nc.declare_dram_parameter("nb", [C, NBANK], f32r, isOutput=False)
    lossraw = nc.declare_dram_parameter(
        "lossraw", [128, M_TILES], f32, isOutput=True
    )

    with tile.TileContext(nc) as tc:
        with (
            tc.tile_pool(name="big", bufs=1) as big,
            tc.tile_pool(name="rows", bufs=4) as rows,
            tc.tile_pool(name="small", bufs=4) as small,
            tc.tile_pool(name="stats", bufs=1) as stats,
            tc.tile_pool(name="psum", bufs=2, space="PSUM") as psum,
        ):
            nb_sb = [big.tile([128, NBANK], f32r, tag=f"nb{k}", name=f"nb_sb{k}") for k in range(2)]
            a_sb = [big.tile([128, R], f32r, tag=f"a{k}", name=f"a_sb{k}") for k in range(2)]
            for k in range(2):
                nc.sync.dma_start(nb_sb[k][:], nb[k * 128 : (k + 1) * 128, :])
            # chan-major anchors in 1024-column chunks so early tiles start sooner
            for k in range(2):
                for q in range(4):
                    cs = slice(q * 1024, (q + 1) * 1024)
                    nc.sync.dma_start(
                        a_sb[k][:, cs], a_cm[k * 128 : (k + 1) * 128, cs]
                    )

            mstat = stats.tile([128, M_TILES], f32)        # -m~ per tile
            sstat = stats.tile([128, 2 * M_TILES], f32)    # per-half exp sums
            postat = stats.tile([128, M_TILES], f32)       # pos_sim (pre-scaled by 2)

            for m in range(M_TILES):
                ms = slice(m * 128, (m + 1) * 128)
                ar = rows.tile([128, C], f32, tag="ar")
                pr = rows.tile([128, C], f32, tag="pr")
                nc.sync.dma_start(ar[:], at_rm[ms, :])
                nc.sync.dma_start(pr[:], pt_rm[ms, :])
                prod = small.tile([128, C], f32, tag="prod")
                nc.vector.scalar_tensor_tensor(
                    out=prod[:],
                    in0=ar[:],
                    scalar=1.0,
                    in1=pr[:],
                    op0=Alu.mult,
                    op1=Alu.mult,
                    accum_out=postat[:, m : m + 1],
                )

                ps = [psum.tile([128, 1024], f32, tag=f"ps{h}", name=f"ps{h}_{m}") for h in range(2)]
                for h in range(2):
                    for j in range(2):
                        for k in range(2):
                            nc.tensor.matmul(
                                ps[h][:, j * 512 : (j + 1) * 512],
                                a_sb[k][:, ms],
                                nb_sb[k][
                                    :, h * 1024 + j * 512 : h * 1024 + (j + 1) * 512
                                ],
                                start=(k == 0),
                                stop=(k == 1),
                            )

                tmpn = small.tile([128, 1], f32, tag="tmpn")
                if _STRIDED_MAX:
                    sub0, sub1 = ps[0][:, ::4], ps[1][:, ::4]
                else:
                    sub0, sub1 = ps[0][:, 0:256], ps[1][:, 0:256]
                nc.vector.reduce_max(mstat[:, m : m + 1], sub0, axis=X, negate=True)
                nc.vector.reduce_max(tmpn[:], sub1, axis=X, negate=True)
                nc.vector.tensor_tensor(
                    mstat[:, m : m + 1], mstat[:, m : m + 1], tmpn[:], op=Alu.min
                )
                for h in range(2):
                    if _EXP_INPLACE:
                        eout = ps[h][:]
                    else:
                        et = small.tile([128, 1024], f32, tag="escr", name=f"escr{h}_{m}")
                        eout = et[:]
                    nc.scalar.activation(
                        eout,
                        ps[h][:],
                        F.Exp,
                        bias=mstat[:, m : m + 1],
                        scale=1.0,
                        accum_out=sstat[:, 2 * m + h : 2 * m + h + 1],
                    )

            # epilogue: frac = u / (u + S*(1+eps)), loss_row = -log(frac+eps)
            sview = sstat[:].rearrange("p (m two) -> p m two", two=2)
            ssum = stats.tile([128, M_TILES], f32)
            nc.vector.tensor_tensor(
                ssum[:], sview[:, :, 0], sview[:, :, 1], op=Alu.add
            )
            t1 = stats.tile([128, M_TILES], f32)
            nc.vector.tensor_tensor(t1[:], postat[:], mstat[:], op=Alu.add)
            u = stats.tile([128, M_TILES], f32)
            nc.scalar.activation(u[:], t1[:], F.Exp)
            den = stats.tile([128, M_TILES], f32)
            nc.vector.scalar_tensor_tensor(
                out=den[:],
                in0=ssum[:],
                scalar=float(1.0 + EPS),
                in1=u[:],
                op0=Alu.mult,
                op1=Alu.add,
            )
            rec = stats.tile([128, M_TILES], f32)
            nc.vector.reciprocal(rec[:], den[:])
            frac = stats.tile([128, M_TILES], f32)
            nc.vector.tensor_tensor(frac[:], u[:], rec[:], op=Alu.mult)
            nc.vector.tensor_scalar_add(frac[:], frac[:], float(EPS))
            lsb = stats.tile([128, M_TILES], f32)
            nc.scalar.activation(lsb[:], frac[:], F.Ln)
            nc.sync.dma_start(lossraw[:], lsb[:])

    nc.compile()
    return nc


def _get_program():
    global _PROGRAM
    if _PROGRAM is None:
        _PROGRAM = _build_program()
    return _PROGRAM


def _reference_fallback(main_out, ema_out, main_label, neg_banks, pos_banks):
    # Exact numpy mirror of the reference; only taken if any patch label
    # mean < 0.1 (never for uniform [0,1) label fills).
    h, w = H // PATCH, W // PATCH
    x = main_out.reshape(B, C, PATCH, h, PATCH, w).transpose(0, 2, 4, 3, 5, 1)
    anchors = x.reshape(B * PATCH * PATCH, h * w, C)
    x = ema_out.reshape(B, C, PATCH, h, PATCH, w).transpose(0, 2, 4, 3, 5, 1)
    pos_pair = x.reshape(B * PATCH * PATCH, h * w, C)
    neg_flat = neg_banks.transpose(0, 2, 3, 1).reshape(-1, C)
    pos_flat = pos_banks.transpose(0, 2, 3, 1).reshape(-1, C)
    hh, ww = 4 * h, 4 * w
    lab = main_label.reshape(B, PATCH, hh, PATCH, ww).mean(axis=(2, 4))
    use_pos = (lab.reshape(-1) < 0.1)[:, None, None]
    sim_neg = np.einsum("pnc,mc->pnm", anchors, neg_flat) / TEMP
    sim_pos = np.einsum("pnc,mc->pnm", anchors, pos_flat) / TEMP
    neg_sim = np.where(use_pos, sim_pos, sim_neg)
    pos_sim = (anchors * pos_pair).sum(-1, keepdims=True) / TEMP
    allsim = np.concatenate([pos_sim, neg_sim], axis=-1)
    m = allsim.max(axis=-1, keepdims=True)
    denom = np.exp(allsim - m).sum(-1) + EPS
    frac = np.exp(pos_sim - m)[..., 0] / denom
    return np.float32(-np.log(frac + EPS).mean())


def kernel(main_out, ema_out, main_label, neg_banks, pos_banks):
    global LAST_EXEC_NS
    main_out = np.asarray(main_out, dtype=np.float32)
    ema_out = np.asarray(ema_out, dtype=np.float32)
    main_label = np.asarray(main_label, dtype=np.float32)
    neg_banks = np.asarray(neg_banks, dtype=np.float32)
    pos_banks = np.asarray(pos_banks, dtype=np.float32)

    h, w = H // PATCH, W // PATCH
    lab = main_label.reshape(B, PATCH, 4 * h, PATCH, 4 * w).mean(axis=(2, 4))
    if (lab < 0.1).any():
        return _reference_fallback(
            main_out, ema_out, main_label, neg_banks, pos_banks
        )

    from concourse.bass_utils import run_bass_kernel_spmd

    nc = _get_program()
    # bank, channel-major [C, L*h*w], pre-scaled by 1/TEMP (exact x2)
    nb2 = np.ascontiguousarray(
        (2.0 * neg_banks).reshape(L, C, h * w).transpose(1, 0, 2).reshape(C, NBANK),
        dtype=np.float32,
    )
    in_maps = []
    for b in range(B):
        A = main_out[b].reshape(C, R)
        in_maps.append(
            {
                "a_cm": A,
                "at_rm": np.ascontiguousarray(A.T),
                "pt_rm": np.ascontiguousarray(ema_out[b].reshape(C, R).T)
                * np.float32(2.0),
                "nb": nb2,
            }
        )

    res = run_bass_kernel_spmd(
        nc, in_maps, list(range(N_CORES)), trace=TRACE
    )
    LAST_EXEC_NS = res.exec_time_ns
    tot = sum(r["lossraw"].astype(np.float64).sum() for r in res.results)
    return np.float32(-(tot / (B * PATCH * PATCH * h * w)))


# revision 8
# speedup vs baseline: 1.0645x; 1.0645x over previous
"""ContrastivePatchLoss TRN2 kernel.

Math (reference): anchors = patches of main_out [512, 64, 256]; sims
against a 2048-entry bank (neg bank normally; pos bank only when a
patch's label-mean < 0.1, which for uniform [0,1) labels is a >40-sigma
event); softmax-style loss vs the ema positive pair; scalar mean.

Sharding: batch element b -> core b (8 cores, 64 patches = 4096 anchor
rows each). Banks replicated. Each core returns its 4096 per-row
log-fracs; host sums and negates.

Per-core pipeline (all engines overlapped, per 128-row tile):
  PE   : sims[128, 2048] = A_chunk.T @ bank (fp32r, 8 matmuls)
  DVE  : negated subsample row-max (stride-4) -> -m~  (safe exp shift)
  ACT  : exp(sims - m~) in-place in PSUM, accum_out = row-sums S
  DVE  : pos_sim via tensor_tensor_reduce on row-major A,2*ema tiles
Epilogue identity: with u = exp(pos - m~),
  frac = u / (u + S*(1+eps))   == exp(pos)/(sum_bank exp(s) * (1+eps) + exp(pos))
which matches the reference's frac with m' = exact bank log-sum-exp;
the only difference vs m=rowmax is the eps*e^m term, a <=~1e-5 relative
perturbation of the denominator. loss_row = -log(frac + eps).
"""

import numpy as np

B, C, H, W = 8, 256, 64, 64
PATCH = 8
TEMP = 0.5
EPS = 1e-5
L = 32
R = H * W            # anchor rows per core (64 patches x 64 positions)
NBANK = L * (H // PATCH) * (W // PATCH)   # 2048
M_TILES = R // 128   # 32
N_CORES = 8

_PROGRAM = None
TRACE = False
LAST_EXEC_NS = None
import os as _os

_MM_DTYPE = _os.environ.get("K_MM", "fp16")       # fp16 | fp32r
_EXPOUT = _os.environ.get("K_EXPOUT", "bf16")     # bf16 | f32


def _build_program():
    import concourse.tile as tile
    from concourse import bacc, mybir

    F = mybir.ActivationFunctionType
    Alu = mybir.AluOpType
    X = mybir.AxisListType.X
    f32 = mybir.dt.float32
    f32r = mybir.dt.float32r
    f16 = mybir.dt.float16
    bf16 = mybir.dt.bfloat16

    use_fp16 = _MM_DTYPE == "fp16"
    mm_dt = f16 if use_fp16 else f32r
    expout_dt = bf16 if _EXPOUT == "bf16" else f32

    nc = bacc.Bacc(None)
    in_dt = f32 if use_fp16 else f32r
    a_cm = nc.declare_dram_parameter("a_cm", [C, R], in_dt, isOutput=False)
    at_rm = nc.declare_dram_parameter("at_rm", [R, C], f32, isOutput=False)
    pt_rm = nc.declare_dram_parameter("pt_rm", [R, C], f32, isOutput=False)
    nb = nc.declare_dram_parameter("nb", [C, NBANK], in_dt, isOutput=False)
    mstat_out = nc.declare_dram_parameter("mstat_out", [128, M_TILES], f32, isOutput=True)
    sstat_out = nc.declare_dram_parameter("sstat_out", [128, M_TILES], f32, isOutput=True)
    postat_out = nc.declare_dram_parameter("postat_out", [128, M_TILES], f32, isOutput=True)

    with tile.TileContext(nc) as tc:
        with (
            tc.tile_pool(name="big", bufs=1) as big,
            tc.tile_pool(name="rows", bufs=4) as rows,
            tc.tile_pool(name="small", bufs=4) as small,
            tc.tile_pool(name="stats", bufs=1) as stats,
            tc.tile_pool(name="psum", bufs=2, space="PSUM") as psum,
        ):
            # raw (DMA-side) tiles and matmul-operand tiles
            nb_sb = [big.tile([128, NBANK], in_dt, tag=f"nb{k}", name=f"nb_sb{k}") for k in range(2)]
            a_sb = [big.tile([128, R], in_dt, tag=f"a{k}", name=f"a_sb{k}") for k in range(2)]
            if use_fp16:
                nb_mm = [big.tile([128, NBANK], f16, tag=f"nbh{k}", name=f"nb_mm{k}") for k in range(2)]
                a_mm = [big.tile([128, R], f16, tag=f"ah{k}", name=f"a_mm{k}") for k in range(2)]
            else:
                nb_mm, a_mm = nb_sb, a_sb

            # bank first (gates every matmul): halves so h0 compute starts early
            for h in range(2):
                hs = slice(h * 1024, (h + 1) * 1024)
                for k in range(2):
                    nc.sync.dma_start(nb_sb[k][:, hs], nb[k * 128 : (k + 1) * 128, hs])
                    if use_fp16:
                        nc.vector.tensor_copy(nb_mm[k][:, hs], nb_sb[k][:, hs])
            # anchors in 1024-column chunks so early tiles start sooner
            for q in range(4):
                cs = slice(q * 1024, (q + 1) * 1024)
                for k in range(2):
                    nc.sync.dma_start(a_sb[k][:, cs], a_cm[k * 128 : (k + 1) * 128, cs])
                    if use_fp16:
                        nc.vector.tensor_copy(a_mm[k][:, cs], a_sb[k][:, cs])

            mstat = stats.tile([128, M_TILES], f32)        # -m~ per tile
            sstat = stats.tile([128, M_TILES], f32)        # bank exp sums
            postat = stats.tile([128, M_TILES], f32)       # pos_sim (pre-scaled by 2)

            for m in range(M_TILES):
                ms = slice(m * 128, (m + 1) * 128)
                ar = rows.tile([128, C], f32, tag="ar")
                pr = rows.tile([128, C], f32, tag="pr")
                nc.sync.dma_start(ar[:], at_rm[ms, :])
                nc.sync.dma_start(pr[:], pt_rm[ms, :])
                prod = small.tile([128, C], f32, tag="prod")
                nc.vector.scalar_tensor_tensor(
                    out=prod[:],
                    in0=ar[:],
                    scalar=1.0,
                    in1=pr[:],
                    op0=Alu.mult,
                    op1=Alu.mult,
                    accum_out=postat[:, m : m + 1],
                )

                ps = psum.tile([128, 2048], f32, tag="ps", name=f"ps_{m}")
                for j in range(4):
                    for k in range(2):
                        nc.tensor.matmul(
                            ps[:, j * 512 : (j + 1) * 512],
                            a_mm[k][:, ms],
                            nb_mm[k][:, j * 512 : (j + 1) * 512],
                            start=(k == 0),
                            stop=(k == 1),
                        )

                nc.vector.reduce_max(
                    mstat[:, m : m + 1], ps[:, ::8], axis=X, negate=True
                )
                escr = small.tile([128, 2048], expout_dt, tag="escr", name=f"escr_{m}")
                nc.scalar.activation(
                    escr[:],
                    ps[:],
                    F.Exp,
                    bias=mstat[:, m : m + 1],
                    scale=1.0,
                    accum_out=sstat[:, m : m + 1],
                )

            nc.sync.dma_start(mstat_out[:], mstat[:])
            nc.sync.dma_start(sstat_out[:], sstat[:])
            nc.sync.dma_start(postat_out[:], postat[:])

    nc.compile()
    return nc


def _get_program():
    global _PROGRAM
    if _PROGRAM is None:
        _PROGRAM = _build_program()
    return _PROGRAM


def _reference_fallback(main_out, ema_out, main_label, neg_banks, pos_banks):
    # Exact numpy mirror of the reference; only taken if any patch label
    # mean < 0.1 (never for uniform [0,1) label fills).
    h, w = H // PATCH, W // PATCH
    x = main_out.reshape(B, C, PATCH, h, PATCH, w).transpose(0, 2, 4, 3, 5, 1)
    anchors = x.reshape(B * PATCH * PATCH, h * w, C)
    x = ema_out.reshape(B, C, PATCH, h, PATCH, w).transpose(0, 2, 4, 3, 5, 1)
    pos_pair = x.reshape(B * PATCH * PATCH, h * w, C)
    neg_flat = neg_banks.transpose(0, 2, 3, 1).reshape(-1, C)
    pos_flat = pos_banks.transpose(0, 2, 3, 1).reshape(-1, C)
    hh, ww = 4 * h, 4 * w
    lab = main_label.reshape(B, PATCH, hh, PATCH, ww).mean(axis=(2, 4))
    use_pos = (lab.reshape(-1) < 0.1)[:, None, None]
    sim_neg = np.einsum("pnc,mc->pnm", anchors, neg_flat) / TEMP
    sim_pos = np.einsum("pnc,mc->pnm", anchors, pos_flat) / TEMP
    neg_sim = np.where(use_pos, sim_pos, sim_neg)
    pos_sim = (anchors * pos_pair).sum(-1, keepdims=True) / TEMP
    allsim = np.concatenate([pos_sim, neg_sim], axis=-1)
    m = allsim.max(axis=-1, keepdims=True)
    denom = np.exp(allsim - m).sum(-1) + EPS
    frac = np.exp(pos_sim - m)[..., 0] / denom
    return np.float32(-np.log(frac + EPS).mean())


def kernel(main_out, ema_out, main_label, neg_banks, pos_banks):
    global LAST_EXEC_NS
    main_out = np.asarray(main_out, dtype=np.float32)
    ema_out = np.asarray(ema_out, dtype=np.float32)
    main_label = np.asarray(main_label, dtype=np.float32)
    neg_banks = np.asarray(neg_banks, dtype=np.float32)
    pos_banks = np.asarray(pos_banks, dtype=np.float32)

    h, w = H // PATCH, W // PATCH
    lab = main_label.reshape(B, PATCH, 4 * h, PATCH, 4 * w).mean(axis=(2, 4))
    if (lab < 0.1).any():
        return _reference_fallback(
            main_out, ema_out, main_label, neg_banks, pos_banks
        )

    from concourse.bass_utils import run_bass_kernel_spmd

    nc = _get_program()
    # bank, channel-major [C, L*h*w], pre-scaled by 1/TEMP (exact x2)
    nb2 = np.ascontiguousarray(
        (2.0 * neg_banks).reshape(L, C, h * w).transpose(1, 0, 2).reshape(C, NBANK),
        dtype=np.float32,
    )
    in_maps = []
    for b in range(B):
        A = main_out[b].reshape(C, R)
        in_maps.append(
            {
                "a_cm": A,
                "at_rm": np.ascontiguousarray(A.T),
                "pt_rm": np.ascontiguousarray(ema_out[b].reshape(C, R).T)
                * np.float32(2.0),
                "nb": nb2,
            }
        )

    res = run_bass_kernel_spmd(
        nc, in_maps, list(range(N_CORES)), trace=TRACE
    )
    LAST_EXEC_NS = res.exec_time_ns
    # fp64 finishing: with m~ <= rowmax and S = sum_bank exp(s - m~),
    # frac = u/(u + S*(1+eps)), u = exp(pos - m~). S=inf rows (subsample
    # max trailed an outlier by >88) correctly collapse to frac=0.
    tot = 0.0
    for r in res.results:
        negm = r["mstat_out"].astype(np.float64)
        S = r["sstat_out"].astype(np.float64)
        pos = r["postat_out"].astype(np.float64)
        u = np.exp(pos + negm)
        frac = u / (u + S * (1.0 + EPS))
        tot += np.log(frac + EPS).sum()
    return np.float32(-(tot / (B * PATCH * PATCH * h * w)))


# revision 10
# speedup vs baseline: 1.1124x; 1.0450x over previous
"""ContrastivePatchLoss TRN2 kernel.

Math (reference): anchors = patches of main_out [512, 64, 256]; sims
against a 2048-entry bank (neg bank normally; pos bank only when a
patch's label-mean < 0.1, which for uniform [0,1) labels is a >40-sigma
event); softmax-style loss vs the ema positive pair; scalar mean.

Sharding: batch element b -> core b (8 cores, 64 patches = 4096 anchor
rows each). Banks replicated. Each core returns its 4096 per-row
log-fracs; host sums and negates.

Per-core pipeline (all engines overlapped, per 128-row tile):
  PE   : sims[128, 2048] = A_chunk.T @ bank (fp32r, 8 matmuls)
  DVE  : negated subsample row-max (stride-4) -> -m~  (safe exp shift)
  ACT  : exp(sims - m~) in-place in PSUM, accum_out = row-sums S
  DVE  : pos_sim via tensor_tensor_reduce on row-major A,2*ema tiles
Epilogue identity: with u = exp(pos - m~),
  frac = u / (u + S*(1+eps))   == exp(pos)/(sum_bank exp(s) * (1+eps) + exp(pos))
which matches the reference's frac with m' = exact bank log-sum-exp;
the only difference vs m=rowmax is the eps*e^m term, a <=~1e-5 relative
perturbation of the denominator. loss_row = -log(frac + eps).
"""

import numpy as np

B, C, H, W = 8, 256, 64, 64
PATCH = 8
TEMP = 0.5
EPS = 1e-5
L = 32
R = H * W            # anchor rows per core (64 patches x 64 positions)
NBANK = L * (H // PATCH) * (W // PATCH)   # 2048
M_TILES = R // 128   # 32
N_CORES = 8

_PROGRAM = None
TRACE = False
LAST_EXEC_NS = None
import os as _os

_MM_DTYPE = _os.environ.get("K_MM", "fp16")       # fp16 | fp32r
_EXPOUT = _os.environ.get("K_EXPOUT", "bf16")     # bf16 | f32


def _build_program():
    import concourse.tile as tile
    from concourse import bacc, mybir

    F = mybir.ActivationFunctionType
    Alu = mybir.AluOpType
    X = mybir.AxisListType.X
    f32 = mybir.dt.float32
    f32r = mybir.dt.float32r
    f16 = mybir.dt.float16
    bf16 = mybir.dt.bfloat16

    use_fp16 = _MM_DTYPE == "fp16"
    mm_dt = f16 if use_fp16 else f32r
    expout_dt = bf16 if _EXPOUT == "bf16" else f32

    nc = bacc.Bacc(None)
    in_dt = f32 if use_fp16 else f32r
    a_cm = nc.declare_dram_parameter("a_cm", [C, R], in_dt, isOutput=False)
    at_rm = nc.declare_dram_parameter("at_rm", [R, C], f32, isOutput=False)
    pt_rm = nc.declare_dram_parameter("pt_rm", [R, C], f32, isOutput=False)
    nb = nc.declare_dram_parameter("nb", [C, NBANK], in_dt, isOutput=False)
    mstat_out = nc.declare_dram_parameter("mstat_out", [128, M_TILES], f32, isOutput=True)
    sstat_out = nc.declare_dram_parameter("sstat_out", [128, M_TILES], f32, isOutput=True)
    postat_out = nc.declare_dram_parameter("postat_out", [128, M_TILES], f32, isOutput=True)

    with tile.TileContext(nc) as tc:
        with (
            tc.tile_pool(name="big", bufs=1) as big,
            tc.tile_pool(name="rows", bufs=4) as rows,
            tc.tile_pool(name="small", bufs=4) as small,
            tc.tile_pool(name="stats", bufs=1) as stats,
            tc.tile_pool(name="psum", bufs=2, space="PSUM") as psum,
        ):
            # raw (DMA-side) tiles and matmul-operand tiles
            nb_sb = [big.tile([128, NBANK], in_dt, tag=f"nb{k}", name=f"nb_sb{k}") for k in range(2)]
            a_sb = [big.tile([128, R], in_dt, tag=f"a{k}", name=f"a_sb{k}") for k in range(2)]
            if use_fp16:
                nb_mm = [big.tile([128, NBANK], f16, tag=f"nbh{k}", name=f"nb_mm{k}") for k in range(2)]
                a_mm = [big.tile([128, R], f16, tag=f"ah{k}", name=f"a_mm{k}") for k in range(2)]
            else:
                nb_mm, a_mm = nb_sb, a_sb

            # PE warm-up: ~16 dummy matmuls on zeroed tiles while DMAs load,
            # so HAM reaches K=8/8 before the first real matmul.
            wz = small.tile([128, 512], f16, tag="warm", name="warmzero")
            nc.gpsimd.memset(wz[:], 0.0)
            wps = psum.tile([128, 512], f32, tag="ps", name="warmps")
            for i in range(16):
                nc.tensor.matmul(wps[:], wz[:, 0:128], wz[:], start=True, stop=True)

            # interleave bank/anchor chunk loads so the first tiles' operands
            # land first: nb h0 -> a q0 -> nb h1 -> a q1..q3
            def load_nb(h):
                hs = slice(h * 1024, (h + 1) * 1024)
                for k in range(2):
                    nc.sync.dma_start(nb_sb[k][:, hs], nb[k * 128 : (k + 1) * 128, hs])
                    if use_fp16:
                        nc.vector.tensor_copy(nb_mm[k][:, hs], nb_sb[k][:, hs])

            def load_a(q):
                cs = slice(q * 1024, (q + 1) * 1024)
                for k in range(2):
                    nc.sync.dma_start(a_sb[k][:, cs], a_cm[k * 128 : (k + 1) * 128, cs])
                    if use_fp16:
                        nc.vector.tensor_copy(a_mm[k][:, cs], a_sb[k][:, cs])

            load_nb(0)
            load_a(0)
            load_nb(1)
            for q in range(1, 4):
                load_a(q)

            mstat = stats.tile([128, M_TILES], f32)        # -m~ per tile
            sstat = stats.tile([128, M_TILES], f32)        # bank exp sums
            postat = stats.tile([128, M_TILES], f32)       # pos_sim (pre-scaled by 2)

            for m in range(M_TILES):
                ms = slice(m * 128, (m + 1) * 128)
                ar = rows.tile([128, C], f32, tag="ar")
                pr = rows.tile([128, C], f32, tag="pr")
                nc.sync.dma_start(ar[:], at_rm[ms, :])
                nc.sync.dma_start(pr[:], pt_rm[ms, :])
                prod = small.tile([128, C], f32, tag="prod")
                nc.vector.scalar_tensor_tensor(
                    out=prod[:],
                    in0=ar[:],
                    scalar=1.0,
                    in1=pr[:],
                    op0=Alu.mult,
                    op1=Alu.mult,
                    accum_out=postat[:, m : m + 1],
                )

                ps = psum.tile([128, 2048], f32, tag="ps", name=f"ps_{m}")
                for j in range(4):
                    for k in range(2):
                        nc.tensor.matmul(
                            ps[:, j * 512 : (j + 1) * 512],
                            a_mm[k][:, ms],
                            nb_mm[k][:, j * 512 : (j + 1) * 512],
                            start=(k == 0),
                            stop=(k == 1),
                        )

                nc.vector.reduce_max(
                    mstat[:, m : m + 1], ps[:, ::16], axis=X, negate=True
                )
                escr = small.tile([128, 2048], expout_dt, tag="escr", name=f"escr_{m}")
                nc.scalar.activation(
                    escr[:],
                    ps[:],
                    F.Exp,
                    bias=mstat[:, m : m + 1],
                    scale=1.0,
                    accum_out=sstat[:, m : m + 1],
                )

            nc.sync.dma_start(mstat_out[:], mstat[:])
            nc.sync.dma_start(sstat_out[:], sstat[:])
            nc.sync.dma_start(postat_out[:], postat[:])

    nc.compile()
    return nc


def _get_program():
    global _PROGRAM
    if _PROGRAM is None:
        _PROGRAM = _build_program()
    return _PROGRAM


def _reference_fallback(main_out, ema_out, main_label, neg_banks, pos_banks):
    # Exact numpy mirror of the reference; only taken if any patch label
    # mean < 0.1 (never for uniform [0,1) label fills).
    h, w = H // PATCH, W // PATCH
    x = main_out.reshape(B, C, PATCH, h, PATCH, w).transpose(0, 2, 4, 3, 5, 1)
    anchors = x.reshape(B * PATCH * PATCH, h * w, C)
    x = ema_out.reshape(B, C, PATCH, h, PATCH, w).transpose(0, 2, 4, 3, 5, 1)
    pos_pair = x.reshape(B * PATCH * PATCH, h * w, C)
    neg_flat = neg_banks.transpose(0, 2, 3, 1).reshape(-1, C)
    pos_flat = pos_banks.transpose(0, 2, 3, 1).reshape(-1, C)
    hh, ww = 4 * h, 4 * w
    lab = main_label.reshape(B, PATCH, hh, PATCH, ww).mean(axis=(2, 4))
    use_pos = (lab.reshape(-1) < 0.1)[:, None, None]
    sim_neg = np.einsum("pnc,mc->pnm", anchors, neg_flat) / TEMP
    sim_pos = np.einsum("pnc,mc->pnm", anchors, pos_flat) / TEMP
    neg_sim = np.where(use_pos, sim_pos, sim_neg)
    pos_sim = (anchors * pos_pair).sum(-1, keepdims=True) / TEMP
    allsim = np.concatenate([pos_sim, neg_sim], axis=-1)
    m = allsim.max(axis=-1, keepdims=True)
    denom = np.exp(allsim - m).sum(-1) + EPS
    frac = np.exp(pos_sim - m)[..., 0] / denom
    return np.float32(-np.log(frac + EPS).mean())


def kernel(main_out, ema_out, main_label, neg_banks, pos_banks):
    global LAST_EXEC_NS
    main_out = np.asarray(main_out, dtype=np.float32)
    ema_out = np.asarray(ema_out, dtype=np.float32)
    main_label = np.asarray(main_label, dtype=np.float32)
    neg_banks = np.asarray(neg_banks, dtype=np.float32)
    pos_banks = np.asarray(pos_banks, dtype=np.float32)

    h, w = H // PATCH, W // PATCH
    lab = main_label.reshape(B, PATCH, 4 * h, PATCH, 4 * w).mean(axis=(2, 4))
    if (lab < 0.1).any():
        return _reference_fallback(
            main_out, ema_out, main_label, neg_banks, pos_banks
        )

    from concourse.bass_utils import run_bass_kernel_spmd

    nc = _get_program()
    # bank, channel-major [C, L*h*w], pre-scaled by 1/TEMP (exact x2)
    nb2 = np.ascontiguousarray(
        (2.0 * neg_banks).reshape(L, C, h * w).transpose(1, 0, 2).reshape(C, NBANK),
        dtype=np.float32,
    )
    in_maps = []
    for b in range(B):
        A = main_out[b].reshape(C, R)
        in_maps.append(
            {
                "a_cm": A,
                "at_rm": np.ascontiguousarray(A.T),
                "pt_rm": np.ascontiguousarray(ema_out[b].reshape(C, R).T)
                * np.float32(2.0),
                "nb": nb2,
            }
        )

    res = run_bass_kernel_spmd(
        nc, in_maps, list(range(N_CORES)), trace=TRACE
    )
    LAST_EXEC_NS = res.exec_time_ns
    # fp64 finishing: with m~ <= rowmax and S = sum_bank exp(s - m~),
    # frac = u/(u + S*(1+eps)), u = exp(pos - m~). S=inf rows (subsample
    # max trailed an outlier by >88) correctly collapse to frac=0.
    tot = 0.0
    for b, r in enumerate(res.results):
        negm = r["mstat_out"].astype(np.float64)
        S = r["sstat_out"].astype(np.float64)
        pos = r["postat_out"].astype(np.float64)
        u = np.exp(pos + negm)
        frac = u / (u + S * (1.0 + EPS))
        lrow = np.log(frac + EPS)
        bad = ~np.isfinite(S)
        if bad.any():
            # S overflowed fp32 (subsample max trailed an outlier by >~88):
            # recompute those rows exactly in fp64 on host.
            A64 = in_maps[b]["a_cm"].astype(np.float64)
            nb64 = nb2.astype(np.float64)
            for p, mt in zip(*np.nonzero(bad)):
                row = mt * 128 + p
                s_row = A64[:, row] @ nb64
                mr = s_row.max()
                Sr = np.exp(s_row - mr).sum()
                ur = np.exp(pos[p, mt] - mr)
                lrow[p, mt] = np.log(ur / (ur + Sr * (1.0 + EPS)) + EPS)
        tot += lrow.sum()
    return np.float32(-(tot / (B * PATCH * PATCH * h * w)))
